# revision 1
# baseline (speedup 1.0000x reference)
"""MoE (top-2 of 8 experts + shared expert) Trainium2 kernel, expert-parallel
across 8 NeuronCores.

Strategy:
  - Host: compute the (tiny) gate in float64 numpy, select top-2 experts per
    token, and dispatch tokens by routing index (the all-to-all of
    expert-parallel MoE, done during the host-side shard step).
  - Work is balanced by slot packing: every core runs three fixed-capacity
    token slots (two routed slots of c1/c2 tokens + one shared-expert slot of
    512 tokens).  Expert token lists are cut into pieces and packed into the
    16 routed slots (exact DP cover), so hot experts span multiple cores and
    every core does identical compute.
  - Device (per core): feature-major MLP per slot. x^T tiles stay resident in
    SBUF; weights stream i-tile by i-tile; swiglu fused into 7 DVE ops + 1 ACT
    op per tile; all matmuls run as float32r (fp32 storage, full-rate PE
    path). h stays resident; the second GEMM accumulates over 16 i-tiles.
  - Host: combine = scatter-add of per-piece outputs weighted by the gate
    probabilities (1.0 for shared slices). The swiglu even/odd interleave
    split, transposes, and the 1/1.702 silu rescale are pre-folded into the
    host-side weight layouts.
"""
import sys

sys.path.insert(0, "/opt/trn_rl_repo")

import numpy as np

import concourse.bacc as bacc_mod
import concourse.tile as tile
from concourse import mybir
from concourse.bass_utils import run_bass_kernel_spmd

F32 = mybir.dt.float32
F32R = mybir.dt.float32r
Alu = mybir.AluOpType
Act = mybir.ActivationFunctionType

ALPHA = 1.702
LIMIT = 7.0
TOPK = 2
D, I, E = 1024, 2048, 8
B, S = 2, 2048
T = B * S
DK = D // 128          # 8 output d-tiles
IT = I // 128          # 16 i-tiles
TS = 512               # shared-expert tokens per core (T / 8)
N_CORES = 8

_kernel_cache = {}


def _token_groups(n):
    """Split n (multiple of 128, >=256) into matmul token groups of <=512,
    each >=256 (float32r needs a moving free dim of at least 256 to run at
    full rate)."""
    groups = []
    rem = n
    while rem > 768:
        groups.append(512)
        rem -= 512
    if rem == 768:
        groups += [512, 256]
    elif rem == 640:
        groups += [384, 256]
    elif rem in (512, 384, 256):
        groups.append(rem)
    else:
        raise ValueError(f"bad token count {n}")
    return groups


def _build(caps):
    """Build the SPMD Bass kernel; caps = token capacity per slot."""
    nc = bacc_mod.Bacc("TRN2")

    def dram(name, shape, dtype=F32R, out=False):
        return nc.declare_dram_parameter(name, list(shape), dtype, isOutput=out)

    slots = []
    for s, cap in enumerate(caps):
        pref = f"s{s}"
        w = {
            "xt": dram(pref + "xt", [DK, 128, cap]),
            "w1e": dram(pref + "w1e", [IT, 128, DK, 128]),
            "w1o": dram(pref + "w1o", [IT, 128, DK, 128]),
            "w3e": dram(pref + "w3e", [IT, 128, DK, 128]),
            "w3o": dram(pref + "w3o", [IT, 128, DK, 128]),
            "w2": dram(pref + "w2", [DK, IT, 128, 128]),
            "b1e": dram(pref + "b1e", [IT, 128], F32),
            "b1o": dram(pref + "b1o", [IT, 128], F32),
            "b3e": dram(pref + "b3e", [IT, 128], F32),
            "b3o": dram(pref + "b3o", [IT, 128], F32),
            "b2": dram(pref + "b2", [DK, 128], F32),
            "y": dram(pref + "y", [DK, 128, cap], F32, out=True),
        }
        slots.append((pref, cap, w))

    with tile.TileContext(nc) as tc:
        with (
            tc.tile_pool(name="persist", bufs=1) as persist,
            tc.tile_pool(name="wpool", bufs=2) as wpool,
            tc.tile_pool(name="work", bufs=2) as work,
            tc.tile_pool(name="outp", bufs=3) as outp,
            tc.tile_pool(name="ps", bufs=1, space="PSUM") as ps,
            tc.tile_pool(name="psy", bufs=3, space="PSUM") as psy,
        ):
            scratch = ps.tile([128, 2], F32, tag="scratch")

            def touch(t):
                # tiny f32r matmul reading only tile t: lets the PE observe
                # t's DMA completion so real matmuls carry at most one wait
                nc.tensor.matmul(scratch[:, :2], t[:, :128], t[:, :2],
                                 start=True, stop=True)

            def phase(pref, t_tot, w):
                groups = _token_groups(t_tot)
                offs = np.cumsum([0] + groups)[:-1]

                xtag = "xt_s0" if pref == "s0" else "xt_s12"
                xts = persist.tile([128, DK * t_tot], F32R, tag=xtag,
                                   name=f"xt_{pref}")
                for dk in range(DK):
                    nc.sync.dma_start(
                        out=xts[:, dk * t_tot:(dk + 1) * t_tot],
                        in_=w["xt"][dk])

                bias = {}
                for bn in ("b1e", "b1o", "b3e", "b3o"):
                    bt = persist.tile([128, IT], F32, tag=bn, name=f"{bn}_{pref}")
                    nc.sync.dma_start(out=bt, in_=w[bn].rearrange("n p -> p n"))
                    bias[bn] = bt
                b2t = persist.tile([128, DK], F32, tag="b2", name=f"b2_{pref}")
                nc.sync.dma_start(out=b2t, in_=w["b2"].rearrange("n p -> p n"))

                htag = "h_a" if pref in ("s0", "s2") else "h_b"
                hbuf = persist.tile([128, IT * t_tot], F32R, tag=htag,
                                    name=f"h_{pref}")

                # ---- first GEMM + swiglu: h[i, t] for all i-tiles ----
                for it in range(IT):
                    ws = {}
                    for wn in ("w1e", "w3e", "w1o", "w3o"):
                        wt = wpool.tile([128, DK * 128], F32R, tag=wn,
                                        name=f"{wn}_{pref}_{it}")
                        nc.sync.dma_start(
                            out=wt.rearrange("p (k i) -> p k i", k=DK),
                            in_=w[wn][it])
                        ws[wn] = wt
                    for g, (goff, gsz) in enumerate(zip(offs, groups)):
                        def mm_acc(tag, wt):
                            acc = ps.tile([128, 512], F32, tag=tag,
                                          name=f"{tag}_{pref}_{it}_{g}")
                            for dk in range(DK):
                                nc.tensor.matmul(
                                    acc[:, :gsz],
                                    wt[:, dk * 128:(dk + 1) * 128],
                                    xts[:, dk * t_tot + goff:
                                        dk * t_tot + goff + gsz],
                                    start=(dk == 0), stop=(dk == DK - 1))
                            return acc

                        A = mm_acc("A", ws["w1e"])
                        Bm = mm_acc("B", ws["w3e"])
                        C = mm_acc("C", ws["w1o"])
                        Dm = mm_acc("D", ws["w3o"])

                        Bp = work.tile([128, 512], F32, tag="Bp")
                        nc.scalar.activation(Bp[:, :gsz], Bm[:, :gsz],
                                             Act.Identity,
                                             bias=bias["b3e"][:, it:it + 1])
                        G = work.tile([128, 512], F32, tag="G")
                        nc.vector.scalar_tensor_tensor(
                            G[:, :gsz], A[:, :gsz], bias["b1e"][:, it:it + 1],
                            Bp[:, :gsz], Alu.add, Alu.mult)
                        nc.vector.tensor_scalar_min(G[:, :gsz], G[:, :gsz], LIMIT)
                        Sg = work.tile([128, 512], F32, tag="Sg")
                        nc.scalar.activation(Sg[:, :gsz], G[:, :gsz],
                                             Act.Sigmoid, scale=ALPHA)
                        # Sv = alpha*G*sigmoid(alpha*G)  (silu(alpha*G))
                        Sv = work.tile([128, 512], F32, tag="Sv")
                        nc.vector.scalar_tensor_tensor(
                            Sv[:, :gsz], G[:, :gsz], ALPHA, Sg[:, :gsz],
                            Alu.mult, Alu.mult)
                        Dp = work.tile([128, 512], F32, tag="Dp")
                        nc.scalar.activation(Dp[:, :gsz], Dm[:, :gsz],
                                             Act.Identity,
                                             bias=bias["b3o"][:, it:it + 1])
                        L = work.tile([128, 512], F32, tag="L")
                        nc.vector.scalar_tensor_tensor(
                            L[:, :gsz], C[:, :gsz], bias["b1o"][:, it:it + 1],
                            Dp[:, :gsz], Alu.add, Alu.mult)
                        nc.vector.tensor_scalar(L[:, :gsz], L[:, :gsz],
                                                LIMIT, -LIMIT, Alu.min, Alu.max)
                        # h = (L + 1) * silu(alpha*G); the 1/alpha rescale is
                        # folded into w2 on the host
                        nc.vector.scalar_tensor_tensor(
                            hbuf[:, it * t_tot + goff: it * t_tot + goff + gsz],
                            L[:, :gsz], 1.0, Sv[:, :gsz], Alu.add, Alu.mult)

                # ---- second GEMM: y[dk] = sum_it w2[dk,it].T @ h[it] ----
                for dk in range(DK):
                    w2t = wpool.tile([128, IT * 128], F32R, tag="w2",
                                     name=f"w2_{pref}_{dk}")
                    nc.sync.dma_start(
                        out=w2t.rearrange("p (n j) -> p n j", n=IT),
                        in_=w["w2"][dk].rearrange("n p j -> p n j"))
                    for g, (goff, gsz) in enumerate(zip(offs, groups)):
                        Y = psy.tile([128, 512], F32, tag="Y",
                                     name=f"Y_{pref}_{dk}_{g}")
                        for it in range(IT):
                            nc.tensor.matmul(
                                Y[:, :gsz],
                                w2t[:, it * 128:(it + 1) * 128],
                                hbuf[:, it * t_tot + goff:
                                     it * t_tot + goff + gsz],
                                start=(it == 0), stop=(it == IT - 1))
                        yo = outp.tile([128, 512], F32, tag="yo")
                        nc.scalar.activation(yo[:, :gsz], Y[:, :gsz],
                                             Act.Identity,
                                             bias=b2t[:, dk:dk + 1])
                        nc.sync.dma_start(
                            out=w["y"][dk, :, goff:goff + gsz],
                            in_=yo[:, :gsz])

            for pref, cap, w in slots:
                phase(pref, cap, w)

    nc.finalize()
    return nc


def _tile_w13(wmat):
    """[D, I] -> [IT, 128, DK, 128] (it, d%128, dk, i%128), contiguous."""
    return np.ascontiguousarray(
        wmat.reshape(DK, 128, IT, 128).transpose(2, 1, 0, 3))


def _tile_w2(wmat):
    """[I, D] -> [DK, IT, 128, 128] (dk, it, i%128, d%128), contiguous."""
    return np.ascontiguousarray(
        wmat.reshape(IT, 128, DK, 128).transpose(2, 0, 1, 3))


def _expert_pack(w1, b1, w3, b3, w2, b2):
    """Split swiglu interleave on the host and tile for DMA."""
    return {
        "w1e": _tile_w13(w1[:, 0::2]),
        "w1o": _tile_w13(w1[:, 1::2]),
        "w3e": _tile_w13(w3[:, 0::2]),
        "w3o": _tile_w13(w3[:, 1::2]),
        "w2": _tile_w2(w2 * np.float32(1.0 / ALPHA)),
        "b1e": np.ascontiguousarray(b1[0::2].reshape(IT, 128)),
        "b1o": np.ascontiguousarray(b1[1::2].reshape(IT, 128)),
        "b3e": np.ascontiguousarray(b3[0::2].reshape(IT, 128)),
        "b3o": np.ascontiguousarray(b3[1::2].reshape(IT, 128)),
        "b2": np.ascontiguousarray(b2.reshape(DK, 128)),
    }


def _xt_pack(xsub, cap):
    """[n, D] tokens -> zero-padded [DK, 128, cap] transposed layout."""
    n = xsub.shape[0]
    xt = np.zeros((D, cap), dtype=np.float32)
    xt[:, :n] = xsub.T
    return np.ascontiguousarray(xt.reshape(DK, 128, cap))


def _pack_slots(counts, c1, c2):
    """Exact DP: cover counts[e] with a1[e] slots of c1 + a2[e] of c2,
    sum(a1) <= 8, sum(a2) <= 8. Returns per-expert (a1, a2) or None."""
    order = np.argsort(-np.asarray(counts))
    opts = []
    for e in order:
        n = counts[e]
        eo = []
        for a1 in range(0, 9):
            need = n - a1 * c1
            a2 = 0 if need <= 0 else -(-need // c2)
            if a2 <= 8:
                eo.append((a1, a2))
                if need <= 0:
                    break
        opts.append(eo)
    memo = {}

    def dp(i, u1, u2):
        if i == len(order):
            return []
        key = (i, u1, u2)
        if key in memo:
            return memo[key]
        res = None
        for a1, a2 in opts[i]:
            if u1 + a1 <= 8 and u2 + a2 <= 8:
                sub = dp(i + 1, u1 + a1, u2 + a2)
                if sub is not None:
                    res = [(a1, a2)] + sub
                    break
        memo[key] = res
        return res

    sol = dp(0, 0, 0)
    if sol is None:
        return None
    out = [None] * len(counts)
    for pos, e in enumerate(order):
        out[e] = sol[pos]
    return out


def kernel(x, gate_w, gate_b, w1, b1, w3, b3, w2, b2,
           sw1, sb1, sw3, sb3, sw2, sb2):
    x = np.asarray(x, dtype=np.float32)
    xt = x.reshape(T, D)

    # ---- gate (float64 host math; selection + combine weights) ----
    z = xt.astype(np.float64) @ np.asarray(gate_w, dtype=np.float64).T
    z -= z.max(axis=-1, keepdims=True)
    ez = np.exp(z)
    scores = ez / ez.sum(axis=-1, keepdims=True)          # [T, E]
    biased = scores + np.asarray(gate_b, dtype=np.float64)
    top2 = np.argsort(-biased, axis=-1, kind="stable")[:, :TOPK]   # [T, 2]
    gate_wt = np.take_along_axis(scores, top2, axis=-1).astype(np.float32)

    tok_idx = []
    tok_wt = []
    for e in range(E):
        sel = np.nonzero((top2 == e).any(axis=1))[0]
        we = np.where(top2[sel, 0] == e, gate_wt[sel, 0], gate_wt[sel, 1])
        tok_idx.append(sel)
        tok_wt.append(we.astype(np.float32))
    counts = [len(s) for s in tok_idx]

    # ---- pack expert token lists into 8x[c1] + 8x[c2] routed slots ----
    c1, c2 = 768, 512
    assign = _pack_slots(counts, c1, c2)
    while assign is None:
        c1 += 128
        assign = _pack_slots(counts, c1, c2)
        if assign is None:
            c2 += 128
            assign = _pack_slots(counts, c1, c2)

    # slot tables: for each core, slot0 (cap c1) and slot1 (cap c2) pieces
    pieces = {0: [], 1: []}                  # slot idx -> list of (e, lo, hi)
    for e in range(E):
        a1, a2 = assign[e]
        lo = 0
        for _ in range(a1):
            hi = min(lo + c1, counts[e])
            pieces[0].append((e, lo, hi))
            lo = hi
        for _ in range(a2):
            hi = min(lo + c2, counts[e])
            pieces[1].append((e, lo, hi))
            lo = hi
        assert lo >= counts[e]
    while len(pieces[0]) < N_CORES:
        pieces[0].append((0, 0, 0))
    while len(pieces[1]) < N_CORES:
        pieces[1].append((0, 0, 0))

    # ---- build per-core input maps ----
    epacks = [
        _expert_pack(np.asarray(w1[e]), np.asarray(b1[e]),
                     np.asarray(w3[e]), np.asarray(b3[e]),
                     np.asarray(w2[e]), np.asarray(b2[e]))
        for e in range(E)
    ]
    spack = _expert_pack(np.asarray(sw1), np.asarray(sb1),
                         np.asarray(sw3), np.asarray(sb3),
                         np.asarray(sw2), np.asarray(sb2))
    caps = (c1, c2, TS)
    in_maps = []
    for c in range(N_CORES):
        m = {}
        for s, cap in ((0, c1), (1, c2)):
            e, lo, hi = pieces[s][c]
            m[f"s{s}xt"] = _xt_pack(xt[tok_idx[e][lo:hi]], cap)
            for k, v in epacks[e].items():
                m[f"s{s}{k}"] = v
        m["s2xt"] = _xt_pack(xt[c * TS:(c + 1) * TS], TS)
        for k, v in spack.items():
            m[f"s2{k}"] = v
        in_maps.append(m)

    # ---- compile (cached) + run on all 8 cores ----
    if caps not in _kernel_cache:
        _kernel_cache[caps] = _build(caps)
    nc = _kernel_cache[caps]
    res = run_bass_kernel_spmd(nc, in_maps, list(range(N_CORES)))

    # ---- combine: weighted scatter-add of routed pieces + shared slices ----
    out = np.zeros((T, D), dtype=np.float32)
    for c in range(N_CORES):
        for s, cap in ((0, c1), (1, c2)):
            e, lo, hi = pieces[s][c]
            if hi <= lo:
                continue
            yc = res.results[c][f"s{s}y"].reshape(D, cap)
            idx = tok_idx[e][lo:hi]
            out[idx] += tok_wt[e][lo:hi][:, None] * yc.T[:hi - lo]
        ysc = res.results[c]["s2y"].reshape(D, TS)
        out[c * TS:(c + 1) * TS] += ysc.T
    return out.reshape(B, S, D)



# revision 4
# speedup vs baseline: 1.4371x; 1.4371x over previous
"""MoE (top-2 of 8 experts + shared expert) Trainium2 kernel, expert-parallel
across 8 NeuronCores.

v2: all-bf16 matmul datapath.

  - Host: gate in float64 numpy; top-2 selection; tokens dispatched by
    routing index during the host-side shard step (the "all-to-all").
  - Work balance: every core runs the same slot structure (a few routed
    slots with fixed token capacities + one shared-expert slot of 512
    tokens). Expert token lists are cut into pieces and packed into the
    slots by an exact-cover DP over slot capacities chosen at runtime to
    minimize total padded capacity (seed-dependent; compiled kernels are
    cached per capacity tuple).
  - Device (per core): feature-major MLP per slot. x^T tiles resident in
    SBUF (bf16); weights stream tile by tile (bf16, separate LDWEIGHTS
    pipelined under the previous matmul); swiglu fused into 6 DVE + 3 ACT
    ops per i-tile; h resident in bf16; second GEMM accumulates over 16
    i-tiles; y written back in bf16.
  - Host: combine = weighted scatter-add of per-piece outputs (1.0 for
    shared slices). The swiglu even/odd interleave split, transposes, and
    the 1/1.702 silu rescale are pre-folded into host-side weight layouts.
"""
import sys

sys.path.insert(0, "/opt/trn_rl_repo")

import itertools
from functools import lru_cache

import ml_dtypes
import numpy as np

import concourse.bacc as bacc_mod
import concourse.tile as tile
from concourse import mybir
from concourse.bass_utils import run_bass_kernel_spmd

F32 = mybir.dt.float32
BF16 = mybir.dt.bfloat16
Alu = mybir.AluOpType
Act = mybir.ActivationFunctionType
NP_BF16 = ml_dtypes.bfloat16

ALPHA = 1.702
LIMIT = 7.0
TOPK = 2
D, I, E = 1024, 2048, 8
B, S = 2, 2048
T = B * S
DK = D // 128          # 8 output d-tiles
IT = I // 128          # 16 i-tiles
TS = 512               # shared-expert tokens per core (T / 8)
N_CORES = 8

_kernel_cache = {}


def _token_groups(cap):
    """Split cap into matmul moving groups, each <=512 (PSUM bank), and
    >=256 where possible (keeps the 107ns LDWEIGHTS hidden under the
    matmul stream)."""
    if cap <= 512:
        return [cap]
    n512, r = divmod(cap, 512)
    if r == 0:
        return [512] * n512
    if r >= 256:
        return [512] * n512 + [r]
    # split the last 512+r into two groups >=256
    a = (512 + r + 1) // 2
    return [512] * (n512 - 1) + [a, 512 + r - a]


def _build(caps):
    """Build the SPMD Bass kernel; caps = token capacity per slot (the
    last slot is the shared-expert slot)."""
    nc = bacc_mod.Bacc("TRN2")

    def dram(name, shape, dtype=BF16, out=False):
        return nc.declare_dram_parameter(name, list(shape), dtype, isOutput=out)

    slots = []
    for s, cap in enumerate(caps):
        pref = f"s{s}"
        w = {
            "xt": dram(pref + "xt", [DK, 128, cap]),
            "w1e": dram(pref + "w1e", [IT, 128, DK, 128]),
            "w1o": dram(pref + "w1o", [IT, 128, DK, 128]),
            "w3e": dram(pref + "w3e", [IT, 128, DK, 128]),
            "w3o": dram(pref + "w3o", [IT, 128, DK, 128]),
            "w2": dram(pref + "w2", [DK, 128, IT, 128]),
            "b1e": dram(pref + "b1e", [128, IT], F32),
            "b1o": dram(pref + "b1o", [128, IT], F32),
            "b3e": dram(pref + "b3e", [128, IT], F32),
            "b3o": dram(pref + "b3o", [128, IT], F32),
            "b2": dram(pref + "b2", [128, DK], F32),
            "y": dram(pref + "y", [DK, 128, cap], BF16, out=True),
        }
        slots.append((pref, cap, w))

    with tile.TileContext(nc) as tc:
        with (
            tc.tile_pool(name="persist", bufs=1) as persist,
            tc.tile_pool(name="wpool", bufs=3) as wpool,
            tc.tile_pool(name="w2pool", bufs=2) as w2pool,
            tc.tile_pool(name="work", bufs=2) as work,
            tc.tile_pool(name="outp", bufs=3) as outp,
            tc.tile_pool(name="ps", bufs=1, space="PSUM") as ps,
            tc.tile_pool(name="psy", bufs=3, space="PSUM") as psy,
        ):
            def phase(pref, cap, w):
                groups = _token_groups(cap)
                offs = np.cumsum([0] + groups)[:-1]

                xts = persist.tile([128, DK * cap], BF16, tag=f"xt_{pref}")
                for dk in range(DK):
                    nc.sync.dma_start(
                        out=xts[:, dk * cap:(dk + 1) * cap],
                        in_=w["xt"][dk])

                bias = {}
                for bn in ("b1e", "b1o", "b3e", "b3o"):
                    bt = persist.tile([128, IT], F32, tag=f"{bn}_{pref}")
                    nc.sync.dma_start(out=bt, in_=w[bn][:, :])
                    bias[bn] = bt
                b2t = persist.tile([128, DK], F32, tag=f"b2_{pref}")
                nc.sync.dma_start(out=b2t, in_=w["b2"][:, :])

                hbuf = persist.tile([128, IT * cap], BF16, tag=f"h_{pref}")

                # ---- first GEMM + swiglu: h[i, t] for all i-tiles ----
                for it in range(IT):
                    ws = {}
                    for wn in ("w1e", "w3e", "w1o", "w3o"):
                        wt = wpool.tile([128, DK * 128], BF16, tag=wn,
                                        name=f"{wn}_{pref}_{it}")
                        nc.sync.dma_start(
                            out=wt.rearrange("p (k i) -> p k i", k=DK),
                            in_=w[wn][it])
                        ws[wn] = wt
                    for g, (goff, gsz) in enumerate(zip(offs, groups)):
                        def mm_acc(tag, wt):
                            acc = ps.tile([128, 512], F32, tag=tag,
                                          name=f"{tag}_{pref}_{it}_{g}")
                            for dk in range(DK):
                                nc.tensor.matmul(
                                    acc[:, :gsz],
                                    wt[:, dk * 128:(dk + 1) * 128],
                                    xts[:, dk * cap + goff:
                                        dk * cap + goff + gsz],
                                    start=(dk == 0), stop=(dk == DK - 1))
                            return acc

                        A = mm_acc("A", ws["w1e"])
                        Bm = mm_acc("B", ws["w3e"])
                        C = mm_acc("C", ws["w1o"])
                        Dm = mm_acc("D", ws["w3o"])

                        Bp = work.tile([128, 512], F32, tag="Bp")
                        nc.scalar.activation(Bp[:, :gsz], Bm[:, :gsz],
                                             Act.Identity,
                                             bias=bias["b3e"][:, it:it + 1])
                        G = work.tile([128, 512], F32, tag="G")
                        nc.vector.scalar_tensor_tensor(
                            G[:, :gsz], A[:, :gsz], bias["b1e"][:, it:it + 1],
                            Bp[:, :gsz], Alu.add, Alu.mult)
                        nc.vector.tensor_scalar_min(G[:, :gsz], G[:, :gsz], LIMIT)
                        Sg = work.tile([128, 512], F32, tag="Sg")
                        nc.scalar.activation(Sg[:, :gsz], G[:, :gsz],
                                             Act.Sigmoid, scale=ALPHA)
                        # Sv = alpha*G*sigmoid(alpha*G)  (silu(alpha*G))
                        Sv = work.tile([128, 512], F32, tag="Sv")
                        nc.vector.scalar_tensor_tensor(
                            Sv[:, :gsz], G[:, :gsz], ALPHA, Sg[:, :gsz],
                            Alu.mult, Alu.mult)
                        Dp = work.tile([128, 512], F32, tag="Dp")
                        nc.scalar.activation(Dp[:, :gsz], Dm[:, :gsz],
                                             Act.Identity,
                                             bias=bias["b3o"][:, it:it + 1])
                        L = work.tile([128, 512], F32, tag="L")
                        nc.vector.scalar_tensor_tensor(
                            L[:, :gsz], C[:, :gsz], bias["b1o"][:, it:it + 1],
                            Dp[:, :gsz], Alu.add, Alu.mult)
                        nc.vector.tensor_scalar(L[:, :gsz], L[:, :gsz],
                                                LIMIT, -LIMIT, Alu.min, Alu.max)
                        # h = (L + 1) * silu(alpha*G); the 1/alpha rescale is
                        # folded into w2 on the host
                        nc.vector.scalar_tensor_tensor(
                            hbuf[:, it * cap + goff: it * cap + goff + gsz],
                            L[:, :gsz], 1.0, Sv[:, :gsz], Alu.add, Alu.mult)

                # ---- second GEMM: y[dk] = sum_it w2[dk,it].T @ h[it] ----
                for dk in range(DK):
                    w2t = w2pool.tile([128, IT * 128], BF16, tag="w2",
                                      name=f"w2_{pref}_{dk}")
                    nc.sync.dma_start(out=w2t, in_=w["w2"][dk])
                    for g, (goff, gsz) in enumerate(zip(offs, groups)):
                        Y = psy.tile([128, 512], F32, tag="Y",
                                     name=f"Y_{pref}_{dk}_{g}")
                        for it in range(IT):
                            nc.tensor.matmul(
                                Y[:, :gsz],
                                w2t[:, it * 128:(it + 1) * 128],
                                hbuf[:, it * cap + goff:
                                     it * cap + goff + gsz],
                                start=(it == 0), stop=(it == IT - 1))
                        yo = outp.tile([128, 512], BF16, tag="yo")
                        nc.scalar.activation(yo[:, :gsz], Y[:, :gsz],
                                             Act.Identity,
                                             bias=b2t[:, dk:dk + 1])
                        nc.sync.dma_start(
                            out=w["y"][dk, :, goff:goff + gsz],
                            in_=yo[:, :gsz])

            for pref, cap, w in slots:
                phase(pref, cap, w)

    nc.finalize()
    return nc


def _tile_w13(wmat):
    """[D, I] -> [IT, 128, DK, 128] (it, d%128, dk, i%128), bf16."""
    return np.ascontiguousarray(
        wmat.reshape(DK, 128, IT, 128).transpose(2, 1, 0, 3).astype(NP_BF16))


def _tile_w2(wmat):
    """[I, D] -> [DK, 128, IT, 128] (dk, i%128, it, d%128), bf16."""
    return np.ascontiguousarray(
        wmat.reshape(IT, 128, DK, 128).transpose(2, 1, 0, 3).astype(NP_BF16))


def _expert_pack(w1, b1, w3, b3, w2, b2):
    """Split swiglu interleave on the host and tile for DMA."""
    return {
        "w1e": _tile_w13(w1[:, 0::2]),
        "w1o": _tile_w13(w1[:, 1::2]),
        "w3e": _tile_w13(w3[:, 0::2]),
        "w3o": _tile_w13(w3[:, 1::2]),
        "w2": _tile_w2(w2 * np.float32(1.0 / ALPHA)),
        "b1e": np.ascontiguousarray(b1[0::2].reshape(IT, 128).T),
        "b1o": np.ascontiguousarray(b1[1::2].reshape(IT, 128).T),
        "b3e": np.ascontiguousarray(b3[0::2].reshape(IT, 128).T),
        "b3o": np.ascontiguousarray(b3[1::2].reshape(IT, 128).T),
        "b2": np.ascontiguousarray(b2.reshape(DK, 128).T),
    }


def _xt_pack(xsub, cap):
    """[n, D] tokens -> zero-padded [DK, 128, cap] transposed bf16."""
    n = xsub.shape[0]
    xt = np.zeros((D, cap), dtype=NP_BF16)
    xt[:, :n] = xsub.T.astype(NP_BF16)
    return np.ascontiguousarray(xt.reshape(DK, 128, cap))


def _pack_scheme(counts, sizes, navail):
    """Exact-cover DP: per expert choose a_j slots of each size so that
    sum_j a_j*sizes[j] >= counts[e], respecting per-size availability.
    Returns per-expert allocation tuples or None."""
    order = sorted(range(len(counts)), key=lambda e: -counts[e])
    K = len(sizes)

    @lru_cache(maxsize=None)
    def dp(i, used):
        if i == len(order):
            return ()
        n = counts[order[i]]
        best = None

        def rec(j, alloc, cap):
            nonlocal best
            if best is not None:
                return
            if cap >= n:
                full = tuple(alloc) + (0,) * (K - len(alloc))
                nu = tuple(u + a for u, a in zip(used, full))
                if all(u <= m for u, m in zip(nu, navail)):
                    sub = dp(i + 1, nu)
                    if sub is not None:
                        best = (full,) + sub
                return
            if j == K:
                return
            for a in range(navail[j] - used[j], -1, -1):
                rec(j + 1, alloc + [a], cap + a * sizes[j])
                if best is not None:
                    return

        rec(0, [], 0)
        return best

    sol = dp(0, (0,) * K)
    if sol is None:
        return None
    out = [None] * len(counts)
    for pos, e in enumerate(order):
        out[e] = sol[pos]
    return out


def _choose_slots(counts):
    """Pick the per-core routed slot-size multiset minimizing total padded
    capacity (tie: fewer slots, then larger minimum size)."""
    size_opts = list(range(128, 513, 64))
    cands = []
    for nslots in (2, 3, 4, 5):
        for combo in itertools.combinations_with_replacement(size_opts, nslots):
            if sum(combo) * N_CORES >= sum(counts):
                cands.append(combo)
    cands.sort(key=lambda c: (sum(c), len(c), -min(c)))
    for combo in cands:
        uniq = sorted(set(combo), reverse=True)
        navail = [N_CORES * combo.count(u) for u in uniq]
        alloc = _pack_scheme(tuple(counts), tuple(uniq), tuple(navail))
        if alloc is not None:
            return combo, uniq, navail, alloc
    raise RuntimeError("no feasible slot scheme")


def kernel(x, gate_w, gate_b, w1, b1, w3, b3, w2, b2,
           sw1, sb1, sw3, sb3, sw2, sb2):
    x = np.asarray(x, dtype=np.float32)
    xt = x.reshape(T, D)

    # ---- gate (float64 host math; selection + combine weights) ----
    z = xt.astype(np.float64) @ np.asarray(gate_w, dtype=np.float64).T
    z -= z.max(axis=-1, keepdims=True)
    ez = np.exp(z)
    scores = ez / ez.sum(axis=-1, keepdims=True)          # [T, E]
    biased = scores + np.asarray(gate_b, dtype=np.float64)
    top2 = np.argsort(-biased, axis=-1, kind="stable")[:, :TOPK]   # [T, 2]
    gate_wt = np.take_along_axis(scores, top2, axis=-1).astype(np.float32)

    tok_idx = []
    tok_wt = []
    for e in range(E):
        sel = np.nonzero((top2 == e).any(axis=1))[0]
        we = np.where(top2[sel, 0] == e, gate_wt[sel, 0], gate_wt[sel, 1])
        tok_idx.append(sel)
        tok_wt.append(we.astype(np.float32))
    counts = [len(s) for s in tok_idx]

    # ---- choose slot scheme + cut experts into pieces ----
    combo, uniq, navail, alloc = _choose_slots(counts)
    # per-core slot list: for each size in combo (sorted desc), one slot
    slot_sizes = sorted(combo, reverse=True)
    # pieces per unique size
    pieces_by_size = {u: [] for u in uniq}
    for e in range(E):
        lo = 0
        for j, u in enumerate(uniq):
            for _ in range(alloc[e][j]):
                hi = min(lo + u, counts[e])
                pieces_by_size[u].append((e, lo, hi))
                lo = hi
        assert lo >= counts[e]
    # assign pieces to slot instances: slot s of the per-core list has size
    # slot_sizes[s]; instance c on core c.
    slot_pieces = []          # [n_slots][n_cores] -> (e, lo, hi)
    used_of_size = {u: 0 for u in uniq}
    for s, u in enumerate(slot_sizes):
        inst = []
        for c in range(N_CORES):
            k = used_of_size[u]
            if k < len(pieces_by_size[u]):
                inst.append(pieces_by_size[u][k])
                used_of_size[u] += 1
            else:
                inst.append((0, 0, 0))
        slot_pieces.append(inst)

    caps = tuple(slot_sizes) + (TS,)

    # ---- build per-core input maps ----
    epacks = [
        _expert_pack(np.asarray(w1[e]), np.asarray(b1[e]),
                     np.asarray(w3[e]), np.asarray(b3[e]),
                     np.asarray(w2[e]), np.asarray(b2[e]))
        for e in range(E)
    ]
    spack = _expert_pack(np.asarray(sw1), np.asarray(sb1),
                         np.asarray(sw3), np.asarray(sb3),
                         np.asarray(sw2), np.asarray(sb2))
    n_routed = len(slot_sizes)
    in_maps = []
    for c in range(N_CORES):
        m = {}
        for s in range(n_routed):
            e, lo, hi = slot_pieces[s][c]
            m[f"s{s}xt"] = _xt_pack(xt[tok_idx[e][lo:hi]], caps[s])
            for k, v in epacks[e].items():
                m[f"s{s}{k}"] = v
        m[f"s{n_routed}xt"] = _xt_pack(xt[c * TS:(c + 1) * TS], TS)
        for k, v in spack.items():
            m[f"s{n_routed}{k}"] = v
        in_maps.append(m)

    # ---- compile (cached) + run on all 8 cores ----
    if caps not in _kernel_cache:
        _kernel_cache[caps] = _build(caps)
    nc = _kernel_cache[caps]
    res = run_bass_kernel_spmd(nc, in_maps, list(range(N_CORES)))

    # ---- combine: weighted scatter-add of routed pieces + shared slices ----
    out = np.zeros((T, D), dtype=np.float32)
    for c in range(N_CORES):
        for s in range(n_routed):
            e, lo, hi = slot_pieces[s][c]
            if hi <= lo:
                continue
            yc = np.asarray(res.results[c][f"s{s}y"],
                            dtype=np.float32).reshape(D, caps[s])
            idx = tok_idx[e][lo:hi]
            out[idx] += tok_wt[e][lo:hi][:, None] * yc.T[:hi - lo]
        ysc = np.asarray(res.results[c][f"s{n_routed}y"],
                         dtype=np.float32).reshape(D, TS)
        out[c * TS:(c + 1) * TS] += ysc.T
    return out.reshape(B, S, D)


# revision 6
# speedup vs baseline: 1.4801x; 1.0299x over previous
"""MoE (top-2 of 8 experts + shared expert) Trainium2 kernel, expert-parallel
across 8 NeuronCores.

v2: all-bf16 matmul datapath.

  - Host: gate in float64 numpy; top-2 selection; tokens dispatched by
    routing index during the host-side shard step (the "all-to-all").
  - Work balance: every core runs the same slot structure (a few routed
    slots with fixed token capacities + one shared-expert slot of 512
    tokens). Expert token lists are cut into pieces and packed into the
    slots by an exact-cover DP over slot capacities chosen at runtime to
    minimize total padded capacity (seed-dependent; compiled kernels are
    cached per capacity tuple).
  - Device (per core): feature-major MLP per slot. x^T tiles resident in
    SBUF (bf16); weights stream tile by tile (bf16, separate LDWEIGHTS
    pipelined under the previous matmul); swiglu fused into 6 DVE + 3 ACT
    ops per i-tile; h resident in bf16; second GEMM accumulates over 16
    i-tiles; y written back in bf16.
  - Host: combine = weighted scatter-add of per-piece outputs (1.0 for
    shared slices). The swiglu even/odd interleave split, transposes, and
    the 1/1.702 silu rescale are pre-folded into host-side weight layouts.
"""
import sys

sys.path.insert(0, "/opt/trn_rl_repo")

import itertools
from functools import lru_cache

import ml_dtypes
import numpy as np

import concourse.bacc as bacc_mod
import concourse.tile as tile
from concourse import mybir
from concourse.bass_utils import run_bass_kernel_spmd

F32 = mybir.dt.float32
BF16 = mybir.dt.bfloat16
Alu = mybir.AluOpType
Act = mybir.ActivationFunctionType
NP_BF16 = ml_dtypes.bfloat16

ALPHA = 1.702
LIMIT = 7.0
TOPK = 2
D, I, E = 1024, 2048, 8
B, S = 2, 2048
T = B * S
DK = D // 128          # 8 output d-tiles
IT = I // 128          # 16 i-tiles
TS = 512               # shared-expert tokens per core (T / 8)
N_CORES = 8

_kernel_cache = {}


def _token_groups(cap):
    """Split cap into matmul moving groups, each <=512 (PSUM bank), and
    >=256 where possible (keeps the 107ns LDWEIGHTS hidden under the
    matmul stream)."""
    if cap <= 512:
        return [cap]
    n512, r = divmod(cap, 512)
    if r == 0:
        return [512] * n512
    if r >= 256:
        return [512] * n512 + [r]
    # split the last 512+r into two groups >=256
    a = (512 + r + 1) // 2
    return [512] * (n512 - 1) + [a, 512 + r - a]


def _build(caps):
    """Build the SPMD Bass kernel; caps = token capacity per slot (the
    last slot is the shared-expert slot)."""
    nc = bacc_mod.Bacc("TRN2")

    def dram(name, shape, dtype=BF16, out=False):
        return nc.declare_dram_parameter(name, list(shape), dtype, isOutput=out)

    slots = []
    for s, cap in enumerate(caps):
        pref = f"s{s}"
        w = {
            "xt": dram(pref + "xt", [DK, 128, cap]),
            "w1e": dram(pref + "w1e", [IT, 128, DK, 128]),
            "w1o": dram(pref + "w1o", [IT, 128, DK, 128]),
            "w3e": dram(pref + "w3e", [IT, 128, DK, 128]),
            "w3o": dram(pref + "w3o", [IT, 128, DK, 128]),
            "w2": dram(pref + "w2", [DK, 128, IT, 128]),
            "b1e": dram(pref + "b1e", [128, IT], F32),
            "b1o": dram(pref + "b1o", [128, IT], F32),
            "b3e": dram(pref + "b3e", [128, IT], F32),
            "b3o": dram(pref + "b3o", [128, IT], F32),
            "b2": dram(pref + "b2", [128, DK], F32),
            "y": dram(pref + "y", [DK, 128, cap], BF16, out=True),
        }
        slots.append((pref, cap, w))

    with tile.TileContext(nc) as tc:
        with (
            tc.tile_pool(name="persist", bufs=1) as persist,
            tc.tile_pool(name="wpool", bufs=4) as wpool,
            tc.tile_pool(name="w2pool", bufs=3) as w2pool,
            tc.tile_pool(name="work", bufs=2) as work,
            tc.tile_pool(name="outp", bufs=3) as outp,
            tc.tile_pool(name="ps", bufs=1, space="PSUM") as ps,
            tc.tile_pool(name="psy", bufs=3, space="PSUM") as psy,
        ):
            def phase(pref, cap, w):
                groups = _token_groups(cap)
                offs = np.cumsum([0] + groups)[:-1]

                def load_w13(it):
                    ws = {}
                    for wn in ("w1e", "w3e", "w1o", "w3o"):
                        wt = wpool.tile([128, DK * 128], BF16, tag=wn,
                                        name=f"{wn}_{pref}_{it}")
                        nc.sync.dma_start(
                            out=wt.rearrange("p (k i) -> p k i", k=DK),
                            in_=w[wn][it])
                        ws[wn] = wt
                    return ws

                # startup-critical order on the sync ring: x^T of the first
                # d-tile, then it0 weights, then the remaining x^T tiles
                xts = persist.tile([128, DK * cap], BF16, tag=f"xt_{pref}")
                nc.sync.dma_start(out=xts[:, 0:cap], in_=w["xt"][0])
                ws0 = load_w13(0)
                for dk in range(1, DK):
                    nc.sync.dma_start(
                        out=xts[:, dk * cap:(dk + 1) * cap],
                        in_=w["xt"][dk])

                # biases feed ACT/DVE only — keep them off the sync ring
                bias = {}
                for bn in ("b1e", "b1o", "b3e", "b3o"):
                    bt = persist.tile([128, IT], F32, tag=f"{bn}_{pref}")
                    nc.gpsimd.dma_start(out=bt, in_=w[bn][:, :])
                    bias[bn] = bt
                b2t = persist.tile([128, DK], F32, tag=f"b2_{pref}")
                nc.gpsimd.dma_start(out=b2t, in_=w["b2"][:, :])

                hbuf = persist.tile([128, IT * cap], BF16, tag=f"h_{pref}")

                # w2 streams on the gpsimd (SWDGE) ring so it is not queued
                # behind the GEMM1 weight stream on the sync ring
                w2tiles = [None] * DK

                def load_w2(dk):
                    t = w2pool.tile([128, IT * 128], BF16, tag="w2",
                                    name=f"w2_{pref}_{dk}")
                    nc.gpsimd.dma_start(out=t, in_=w["w2"][dk])
                    w2tiles[dk] = t

                # ---- first GEMM + swiglu: h[i, t] for all i-tiles ----
                for it in range(IT):
                    ws = ws0 if it == 0 else load_w13(it)
                    if it == IT - 2:
                        load_w2(0)
                    elif it == IT - 1:
                        load_w2(1)
                    for g, (goff, gsz) in enumerate(zip(offs, groups)):
                        def mm_acc(tag, wt):
                            acc = ps.tile([128, 512], F32, tag=tag,
                                          name=f"{tag}_{pref}_{it}_{g}")
                            for dk in range(DK):
                                nc.tensor.matmul(
                                    acc[:, :gsz],
                                    wt[:, dk * 128:(dk + 1) * 128],
                                    xts[:, dk * cap + goff:
                                        dk * cap + goff + gsz],
                                    start=(dk == 0), stop=(dk == DK - 1))
                            return acc

                        A = mm_acc("A", ws["w1e"])
                        Bm = mm_acc("B", ws["w3e"])
                        C = mm_acc("C", ws["w1o"])
                        Dm = mm_acc("D", ws["w3o"])

                        Bp = work.tile([128, 512], F32, tag="Bp")
                        nc.scalar.activation(Bp[:, :gsz], Bm[:, :gsz],
                                             Act.Identity,
                                             bias=bias["b3e"][:, it:it + 1])
                        G = work.tile([128, 512], F32, tag="G")
                        nc.vector.scalar_tensor_tensor(
                            G[:, :gsz], A[:, :gsz], bias["b1e"][:, it:it + 1],
                            Bp[:, :gsz], Alu.add, Alu.mult)
                        nc.vector.tensor_scalar_min(G[:, :gsz], G[:, :gsz], LIMIT)
                        Sg = work.tile([128, 512], F32, tag="Sg")
                        nc.scalar.activation(Sg[:, :gsz], G[:, :gsz],
                                             Act.Sigmoid, scale=ALPHA)
                        # Sv = alpha*G*sigmoid(alpha*G)  (silu(alpha*G))
                        Sv = work.tile([128, 512], F32, tag="Sv")
                        nc.vector.scalar_tensor_tensor(
                            Sv[:, :gsz], G[:, :gsz], ALPHA, Sg[:, :gsz],
                            Alu.mult, Alu.mult)
                        Dp = work.tile([128, 512], F32, tag="Dp")
                        nc.scalar.activation(Dp[:, :gsz], Dm[:, :gsz],
                                             Act.Identity,
                                             bias=bias["b3o"][:, it:it + 1])
                        L = work.tile([128, 512], F32, tag="L")
                        nc.vector.scalar_tensor_tensor(
                            L[:, :gsz], C[:, :gsz], bias["b1o"][:, it:it + 1],
                            Dp[:, :gsz], Alu.add, Alu.mult)
                        nc.vector.tensor_scalar(L[:, :gsz], L[:, :gsz],
                                                LIMIT, -LIMIT, Alu.min, Alu.max)
                        # h = (L + 1) * silu(alpha*G); the 1/alpha rescale is
                        # folded into w2 on the host
                        nc.vector.scalar_tensor_tensor(
                            hbuf[:, it * cap + goff: it * cap + goff + gsz],
                            L[:, :gsz], 1.0, Sv[:, :gsz], Alu.add, Alu.mult)

                # ---- second GEMM: y[dk] = sum_it w2[dk,it].T @ h[it] ----
                for dk in range(DK):
                    if dk + 2 < DK:
                        load_w2(dk + 2)
                    w2t = w2tiles[dk]
                    for g, (goff, gsz) in enumerate(zip(offs, groups)):
                        Y = psy.tile([128, 512], F32, tag="Y",
                                     name=f"Y_{pref}_{dk}_{g}")
                        for it in range(IT):
                            nc.tensor.matmul(
                                Y[:, :gsz],
                                w2t[:, it * 128:(it + 1) * 128],
                                hbuf[:, it * cap + goff:
                                     it * cap + goff + gsz],
                                start=(it == 0), stop=(it == IT - 1))
                        yo = outp.tile([128, 512], BF16, tag="yo")
                        nc.scalar.activation(yo[:, :gsz], Y[:, :gsz],
                                             Act.Identity,
                                             bias=b2t[:, dk:dk + 1])
                        nc.scalar.dma_start(
                            out=w["y"][dk, :, goff:goff + gsz],
                            in_=yo[:, :gsz])

            for pref, cap, w in slots:
                phase(pref, cap, w)

    nc.finalize()
    return nc


def _tile_w13(wmat):
    """[D, I] -> [IT, 128, DK, 128] (it, d%128, dk, i%128), bf16."""
    return np.ascontiguousarray(
        wmat.reshape(DK, 128, IT, 128).transpose(2, 1, 0, 3).astype(NP_BF16))


def _tile_w2(wmat):
    """[I, D] -> [DK, 128, IT, 128] (dk, i%128, it, d%128), bf16."""
    return np.ascontiguousarray(
        wmat.reshape(IT, 128, DK, 128).transpose(2, 1, 0, 3).astype(NP_BF16))


def _expert_pack(w1, b1, w3, b3, w2, b2):
    """Split swiglu interleave on the host and tile for DMA."""
    return {
        "w1e": _tile_w13(w1[:, 0::2]),
        "w1o": _tile_w13(w1[:, 1::2]),
        "w3e": _tile_w13(w3[:, 0::2]),
        "w3o": _tile_w13(w3[:, 1::2]),
        "w2": _tile_w2(w2 * np.float32(1.0 / ALPHA)),
        "b1e": np.ascontiguousarray(b1[0::2].reshape(IT, 128).T),
        "b1o": np.ascontiguousarray(b1[1::2].reshape(IT, 128).T),
        "b3e": np.ascontiguousarray(b3[0::2].reshape(IT, 128).T),
        "b3o": np.ascontiguousarray(b3[1::2].reshape(IT, 128).T),
        "b2": np.ascontiguousarray(b2.reshape(DK, 128).T),
    }


def _xt_pack(xsub, cap):
    """[n, D] tokens -> zero-padded [DK, 128, cap] transposed bf16."""
    n = xsub.shape[0]
    xt = np.zeros((D, cap), dtype=NP_BF16)
    xt[:, :n] = xsub.T.astype(NP_BF16)
    return np.ascontiguousarray(xt.reshape(DK, 128, cap))


def _pack_scheme(counts, sizes, navail):
    """Exact-cover DP: per expert choose a_j slots of each size so that
    sum_j a_j*sizes[j] >= counts[e], respecting per-size availability.
    Returns per-expert allocation tuples or None."""
    order = sorted(range(len(counts)), key=lambda e: -counts[e])
    K = len(sizes)

    @lru_cache(maxsize=None)
    def dp(i, used):
        if i == len(order):
            return ()
        n = counts[order[i]]
        best = None

        def rec(j, alloc, cap):
            nonlocal best
            if best is not None:
                return
            if cap >= n:
                full = tuple(alloc) + (0,) * (K - len(alloc))
                nu = tuple(u + a for u, a in zip(used, full))
                if all(u <= m for u, m in zip(nu, navail)):
                    sub = dp(i + 1, nu)
                    if sub is not None:
                        best = (full,) + sub
                return
            if j == K:
                return
            for a in range(navail[j] - used[j], -1, -1):
                rec(j + 1, alloc + [a], cap + a * sizes[j])
                if best is not None:
                    return

        rec(0, [], 0)
        return best

    sol = dp(0, (0,) * K)
    if sol is None:
        return None
    out = [None] * len(counts)
    for pos, e in enumerate(order):
        out[e] = sol[pos]
    return out


def _choose_slots(counts):
    """Pick the per-core routed slot-size multiset minimizing total padded
    capacity (tie: fewer slots, then larger minimum size)."""
    size_opts = list(range(128, 513, 64))
    cands = []
    for nslots in (2, 3, 4, 5):
        for combo in itertools.combinations_with_replacement(size_opts, nslots):
            if sum(combo) * N_CORES >= sum(counts):
                cands.append(combo)
    cands.sort(key=lambda c: (sum(c), len(c), -min(c)))
    for combo in cands:
        uniq = sorted(set(combo), reverse=True)
        navail = [N_CORES * combo.count(u) for u in uniq]
        alloc = _pack_scheme(tuple(counts), tuple(uniq), tuple(navail))
        if alloc is not None:
            return combo, uniq, navail, alloc
    raise RuntimeError("no feasible slot scheme")


def kernel(x, gate_w, gate_b, w1, b1, w3, b3, w2, b2,
           sw1, sb1, sw3, sb3, sw2, sb2):
    x = np.asarray(x, dtype=np.float32)
    xt = x.reshape(T, D)

    # ---- gate (float64 host math; selection + combine weights) ----
    z = xt.astype(np.float64) @ np.asarray(gate_w, dtype=np.float64).T
    z -= z.max(axis=-1, keepdims=True)
    ez = np.exp(z)
    scores = ez / ez.sum(axis=-1, keepdims=True)          # [T, E]
    biased = scores + np.asarray(gate_b, dtype=np.float64)
    top2 = np.argsort(-biased, axis=-1, kind="stable")[:, :TOPK]   # [T, 2]
    gate_wt = np.take_along_axis(scores, top2, axis=-1).astype(np.float32)

    tok_idx = []
    tok_wt = []
    for e in range(E):
        sel = np.nonzero((top2 == e).any(axis=1))[0]
        we = np.where(top2[sel, 0] == e, gate_wt[sel, 0], gate_wt[sel, 1])
        tok_idx.append(sel)
        tok_wt.append(we.astype(np.float32))
    counts = [len(s) for s in tok_idx]

    # ---- choose slot scheme + cut experts into pieces ----
    combo, uniq, navail, alloc = _choose_slots(counts)
    # per-core slot list: for each size in combo (sorted desc), one slot
    slot_sizes = sorted(combo, reverse=True)
    # pieces per unique size
    pieces_by_size = {u: [] for u in uniq}
    for e in range(E):
        lo = 0
        for j, u in enumerate(uniq):
            for _ in range(alloc[e][j]):
                hi = min(lo + u, counts[e])
                pieces_by_size[u].append((e, lo, hi))
                lo = hi
        assert lo >= counts[e]
    # assign pieces to slot instances: slot s of the per-core list has size
    # slot_sizes[s]; instance c on core c.
    slot_pieces = []          # [n_slots][n_cores] -> (e, lo, hi)
    used_of_size = {u: 0 for u in uniq}
    for s, u in enumerate(slot_sizes):
        inst = []
        for c in range(N_CORES):
            k = used_of_size[u]
            if k < len(pieces_by_size[u]):
                inst.append(pieces_by_size[u][k])
                used_of_size[u] += 1
            else:
                inst.append((0, 0, 0))
        slot_pieces.append(inst)

    caps = tuple(slot_sizes) + (TS,)

    # ---- build per-core input maps ----
    epacks = [
        _expert_pack(np.asarray(w1[e]), np.asarray(b1[e]),
                     np.asarray(w3[e]), np.asarray(b3[e]),
                     np.asarray(w2[e]), np.asarray(b2[e]))
        for e in range(E)
    ]
    spack = _expert_pack(np.asarray(sw1), np.asarray(sb1),
                         np.asarray(sw3), np.asarray(sb3),
                         np.asarray(sw2), np.asarray(sb2))
    n_routed = len(slot_sizes)
    in_maps = []
    for c in range(N_CORES):
        m = {}
        for s in range(n_routed):
            e, lo, hi = slot_pieces[s][c]
            m[f"s{s}xt"] = _xt_pack(xt[tok_idx[e][lo:hi]], caps[s])
            for k, v in epacks[e].items():
                m[f"s{s}{k}"] = v
        m[f"s{n_routed}xt"] = _xt_pack(xt[c * TS:(c + 1) * TS], TS)
        for k, v in spack.items():
            m[f"s{n_routed}{k}"] = v
        in_maps.append(m)

    # ---- compile (cached) + run on all 8 cores ----
    if caps not in _kernel_cache:
        _kernel_cache[caps] = _build(caps)
    nc = _kernel_cache[caps]
    res = run_bass_kernel_spmd(nc, in_maps, list(range(N_CORES)))

    # ---- combine: weighted scatter-add of routed pieces + shared slices ----
    out = np.zeros((T, D), dtype=np.float32)
    for c in range(N_CORES):
        for s in range(n_routed):
            e, lo, hi = slot_pieces[s][c]
            if hi <= lo:
                continue
            yc = np.asarray(res.results[c][f"s{s}y"],
                            dtype=np.float32).reshape(D, caps[s])
            idx = tok_idx[e][lo:hi]
            out[idx] += tok_wt[e][lo:hi][:, None] * yc.T[:hi - lo]
        ysc = np.asarray(res.results[c][f"s{n_routed}y"],
                         dtype=np.float32).reshape(D, TS)
        out[c * TS:(c + 1) * TS] += ysc.T
    return out.reshape(B, S, D)
